# revision 18
# baseline (speedup 1.0000x reference)
"""DenseEdgeConv (gnn_message_passing) Bass kernel for 8 TRN2 NeuronCores.

Model (B=4, N=4096, D=64, K=16, G=64, L=4):
  knn_idx = 16-NN of pos within each cloud (excluding self)
  edge MLP: 4 dense layers over [x_i, x_j, x_j - x_i] with dense (concat) growth
  out = max over neighbors of [r4, r3, r2, r1, x_i]   -> (B, N, 320)

Sharding: 8 cores = (batch b, query-half h); each core handles 2048 queries of
one cloud with the full cloud replicated (KNN is within-cloud).

Per core: 8 pairs of 128-query tiles, software-pipelined 2 deep so the DVE
selection of pair p overlaps the PE MLP of pair p-2 and gpsimd aggregation of
p-2; engine queues stay dense instead of ping-ponging.

  Selection per tile: PE computes s = 2q.c - |c|^2 - |q|^2 = -|q-c|^2 (+~6e-6)
  with a K=30 bf16 triple-split matmul; a -2^20 identity matmul accumulated on
  the self block removes self. ACT copies PSUM->SBUF. DVE takes top-8 per
  512-chunk (max8), recovers within-chunk positions (max_index), packs the
  9-bit position into the low mantissa bits of the score (monotone for the
  all-negative scores; costs 2^-14 relative quantization, validated 23/32768
  wrong neighbor sets on the actual data), merges to the top-16 with
  max8+match_replace on the packed values, and recovers each winner's chunk
  with a 64-wide max_index. Global ids are rebuilt with two ALU ops - no
  full-row index scan.

  MLP per pair: neighbor ids are PE-transposed into gpsimd's 16-wrapped
  layout, ap_gather pulls neighbor feature columns, and blockdiag-packed f32r
  matmuls (two 512-token folds per instruction) run the 4 layers with
  per-point terms via step-0 broadcast APs; ACT applies bias+relu from PSUM.
  The 16-neighbor max aggregation runs as a 4-round tournament on gpsimd
  (DVE is the critical engine; gpsimd has slack).
"""

import contextlib
import dataclasses

import ml_dtypes
import numpy as np

import concourse.bacc as bacc
import concourse.mybir as mybir
import concourse.tile as tile
from concourse import bass_utils

B, N, D, K16, G = 4, 4096, 64, 16, 64
NQ = N // 2            # queries per core
NTILE = NQ // 128      # 16 query tiles per core
NPAIR = NTILE // 2     # 8 tile pairs
FT = 256 * K16 // 2    # 2048 folded columns per pair (4096 tokens)
CH = 512               # selection chunk size
NCH = N // CH          # 8 chunks
LB = 9                 # position bits packed into score mantissa
OUTF = D + 4 * G       # 320 output features
KAUG = 30              # bf16 triple-split score lanes

f32 = mybir.dt.float32
f32r = mybir.dt.float32r
bf16 = mybir.dt.bfloat16
u32 = mybir.dt.uint32
i16 = mybir.dt.int16


def _as_dt(ap, dt):
    t = dataclasses.replace(ap.tensor, dtype=dt)
    return dataclasses.replace(ap, tensor=t)


def _stride2(ap, n, off):
    # view [p, 2n] as [p, n] with step 2, starting at element `off`
    return dataclasses.replace(
        ap, offset=ap.offset + off, ap=type(ap.ap)([list(ap.ap[0]), [2, n]])
    )


def _bcast16(ap, cols):
    # [p, cols] slice -> [p, cols, 16] with step-0 inner dim (16x per-query repeat)
    return dataclasses.replace(
        ap, ap=type(ap.ap)([list(ap.ap[0]), [1, cols], [0, 16]])
    )


def _out3d(ap):
    # d_out[0:256, c0:c0+128] -> [64 rows, 4 layer-blocks, 128 cols] view
    return dataclasses.replace(
        ap, ap=type(ap.ap)([[N // 2, 64], [64 * (N // 2), 4], [1, 128]])
    )


def _r4d(ap):
    # [128, 4*FT] r tile -> [128, 4 layers, 128 queries, 16 neighbors]
    return dataclasses.replace(
        ap, ap=type(ap.ap)([list(ap.ap[0]), [FT, 4], [16, 128], [1, 16]])
    )


def build_nc():
    nc = bacc.Bacc(None, target_bir_lowering=False)

    d_caug = nc.dram_tensor("caug", [KAUG, N], bf16, kind="ExternalInput")
    d_qaug = nc.dram_tensor("qaug", [KAUG, NQ], bf16, kind="ExternalInput")
    d_xtf = nc.dram_tensor("xtf", [128, N], f32, kind="ExternalInput")
    d_xtqf = nc.dram_tensor("xtqf", [128, NQ // 2], f32r, kind="ExternalInput")
    d_xtq = nc.dram_tensor("xtq", [D, NQ], f32, kind="ExternalInput")
    WNAMES = ["w1b", "w1a", "w2r1", "w2x", "w3r2", "w3r1", "w3x",
              "w4r3", "w4r2", "w4r1", "w4x"]
    d_w = {n: nc.dram_tensor(n, [128, 128], f32 if n == "w1b" else f32r,
                             kind="ExternalInput") for n in WNAMES}
    d_b = {l: nc.dram_tensor(f"b{l}", [128, 1], f32, kind="ExternalInput")
           for l in (1, 2, 3, 4)}
    d_ident = nc.dram_tensor("ident", [128, 128], f32, kind="ExternalInput")
    d_nbig = nc.dram_tensor("nbig", [128, 128], bf16, kind="ExternalInput")
    d_selfb = nc.dram_tensor("selfb", [128, 2048], bf16, kind="ExternalInput")
    d_ic = nc.dram_tensor("intc", [128, 112], u32, kind="ExternalInput")
    d_out = nc.dram_tensor("out", [OUTF, NQ], f32, kind="ExternalOutput")

    with tile.TileContext(nc) as tc:
        ctx = contextlib.ExitStack()
        with ctx:
            const = ctx.enter_context(tc.tile_pool(name="const", bufs=1))
            t_caug = const.tile([KAUG, N], bf16)
            t_qaug = const.tile([KAUG, NQ], bf16)
            t_xtf = const.tile([128, N], f32)
            t_xtqf = const.tile([128, NQ // 2], f32r)
            t_w = {n: const.tile([128, 128], f32 if n == "w1b" else f32r,
                                 tag=f"w_{n}", name=f"w_{n}") for n in WNAMES}
            t_b = {l: const.tile([128, 1], f32, tag=f"b_{l}", name=f"b_{l}")
                   for l in (1, 2, 3, 4)}
            t_ident = const.tile([128, 128], f32)
            t_nbig = const.tile([128, 128], bf16)
            t_selfb = const.tile([128, 2048], bf16)
            t_ic = const.tile([128, 112], u32)
            for dst, src in ((t_caug, d_caug), (t_qaug, d_qaug),
                             (t_xtf, d_xtf), (t_xtqf, d_xtqf),
                             (t_ident, d_ident), (t_nbig, d_nbig),
                             (t_selfb, d_selfb), (t_ic, d_ic)):
                nc.sync.dma_start(dst[:], src[:])
            for n in WNAMES:
                nc.sync.dma_start(t_w[n][:], d_w[n][:])
            for l in (1, 2, 3, 4):
                nc.sync.dma_start(t_b[l][:], d_b[l][:])
            # x_i part of the output passes straight through
            nc.sync.dma_start(d_out[4 * G:OUTF, :], d_xtq[:])

            psd = ctx.enter_context(tc.tile_pool(name="psd", bufs=2, space="PSUM"))
            psm = ctx.enter_context(tc.tile_pool(name="psm", bufs=2, space="PSUM"))
            pst = ctx.enter_context(tc.tile_pool(name="pst", bufs=2, space="PSUM"))
            d2p = ctx.enter_context(tc.tile_pool(name="d2p", bufs=4))
            selp = ctx.enter_context(tc.tile_pool(name="selp", bufs=2))
            sp = ctx.enter_context(tc.tile_pool(name="sp", bufs=2))
            idxp = ctx.enter_context(tc.tile_pool(name="idxp", bufs=2))
            xgp = ctx.enter_context(tc.tile_pool(name="xgp", bufs=3))
            rp = ctx.enter_context(tc.tile_pool(name="rp", bufs=2))
            aggp = ctx.enter_context(tc.tile_pool(name="aggp", bufs=2))

            relu = mybir.ActivationFunctionType.Relu
            ident_f = mybir.ActivationFunctionType.Identity
            mx = mybir.AluOpType.max
            alu = mybir.AluOpType

            S_t, IDX_t, XG_t, R_t, AGG_t = {}, {}, {}, {}, {}

            D2_t = {}

            def stage_scores(p):
                for sub in range(2):
                    t = 2 * p + sub
                    # host rolls the candidate order by each core's query
                    # offset, so tile t's self block is always at 128*t
                    c_self = 128 * t
                    t_d2 = d2p.tile([128, N], f32, tag="d2sb", name="d2sb")
                    D2_t[t] = t_d2
                    for quarter in range(4):
                        p_d2 = psd.tile([128, 1024], f32, tag="psd2", name="psd2")
                        c0 = quarter * 1024
                        for j in range(2):
                            lo = c0 + j * 512
                            has_self = lo <= c_self < lo + 512
                            nc.tensor.matmul(
                                p_d2[:, j * 512:(j + 1) * 512],
                                t_qaug[:, t * 128:(t + 1) * 128],
                                t_caug[:, lo:lo + 512],
                                start=True, stop=not has_self)
                            if has_self:
                                v = t % 4    # identity offset within the half
                                nc.tensor.matmul(
                                    p_d2[:, j * 512:(j + 1) * 512],
                                    t_nbig[:],
                                    t_selfb[:, 512 * v:512 * v + 512],
                                    start=False, stop=True)
                        nc.scalar.copy(t_d2[:, c0:c0 + 1024], p_d2[:])

            def stage_sel(p):
                t_S = sp.tile([128, 128], f32, tag="S", name="S")
                S_t[p] = t_S
                for sub in range(2):
                    t = 2 * p + sub
                    t_d2 = D2_t.pop(t)
                    # L1: top-8 values + within-chunk positions per 512-chunk
                    t_V = selp.tile([128, 8 * NCH], f32, tag="V", name="V")
                    t_I = selp.tile([128, 8 * NCH], u32, tag="I", name="I")
                    for c in range(NCH):
                        nc.vector.max(t_V[:, 8 * c:8 * c + 8],
                                      t_d2[:, CH * c:CH * (c + 1)])
                    for c in range(NCH):
                        nc.vector.max_index(t_I[:, 8 * c:8 * c + 8],
                                            t_V[:, 8 * c:8 * c + 8],
                                            t_d2[:, CH * c:CH * (c + 1)])
                    # pack positions into the low LB mantissa bits
                    t_P = selp.tile([128, 8 * NCH], f32, tag="P", name="P")
                    nc.vector.tensor_tensor(
                        _as_dt(t_P[:], u32), _as_dt(t_V[:], u32),
                        t_ic[:, 0:64], op=alu.bitwise_and)
                    nc.vector.tensor_tensor(
                        _as_dt(t_P[:], u32), _as_dt(t_P[:], u32), t_I[:],
                        op=alu.bitwise_or)
                    # merge to top-16 packed + candidate slots
                    t_v16 = selp.tile([128, 16], f32, tag="v16", name="v16")
                    t_pos = selp.tile([128, 16], u32, tag="pos", name="pos")
                    nc.vector.max(t_v16[:, 0:8], t_P[:])
                    nc.vector.max_index(t_pos[:, 0:8], t_v16[:, 0:8], t_P[:])
                    nc.vector.match_replace(
                        t_P[:], in_to_replace=t_v16[:, 0:8], in_values=t_P[:],
                        imm_value=-1e30)
                    nc.vector.max(t_v16[:, 8:16], t_P[:])
                    nc.vector.max_index(t_pos[:, 8:16], t_v16[:, 8:16], t_P[:])
                    # global id = (slot>>3)*CH + (packed & (2^LB-1))
                    t_cb = selp.tile([128, 16], u32, tag="cb", name="cb")
                    nc.vector.tensor_tensor(
                        t_cb[:], t_pos[:], t_ic[:, 80:96],
                        op=alu.logical_shift_right)
                    nc.vector.tensor_tensor(
                        t_cb[:], t_cb[:], t_ic[:, 96:112],
                        op=alu.logical_shift_left)
                    t_g = selp.tile([128, 16], u32, tag="g", name="g")
                    nc.vector.tensor_tensor(
                        t_g[:], _as_dt(t_v16[:], u32), t_ic[:, 64:80],
                        op=alu.bitwise_and)
                    nc.vector.tensor_tensor(
                        t_g[:], t_g[:], t_cb[:], op=alu.add)
                    # write ids into S (as f32) and replicate 3x for the
                    # per-16-partition gather wrap
                    s_blk = t_S[:, 64 * sub:64 * sub + 16]
                    nc.vector.tensor_copy(s_blk, t_g[:])
                    rep_out = dataclasses.replace(
                        t_S[:, 64 * sub + 16:64 * sub + 64],
                        ap=type(s_blk.ap)([list(s_blk.ap[0]), [16, 3], [1, 16]]))
                    rep_in = dataclasses.replace(
                        s_blk, ap=type(s_blk.ap)([list(s_blk.ap[0]), [0, 3], [1, 16]]))
                    nc.vector.tensor_copy(rep_out, rep_in)

            PT_t = {}

            def stage_transpose(p):
                p_T = pst.tile([128, 128], f32, tag="ptr", name="ptr")
                PT_t[p] = p_T
                nc.tensor.transpose(p_T[:], S_t.pop(p)[:], t_ident[:])

            def stage_cidx(p):
                # PSUM->SBUF i16 cast on the ACT queue: keeps the DVE free of
                # any same-iteration PE dependency
                t_IDX = idxp.tile([128, 128], i16, tag="IDX", name="IDX")
                IDX_t[p] = t_IDX
                nc.scalar.copy(t_IDX[:], PT_t.pop(p)[:])

            def stage_gather(p):
                t_xg = xgp.tile([128, FT], f32, tag="xg", name="xg")
                XG_t[p] = t_xg
                nc.gpsimd.ap_gather(
                    t_xg[:].rearrange("c (n d) -> c n d", d=1),
                    t_xtf[:].rearrange("c (n d) -> c n d", d=1),
                    IDX_t.pop(p)[:],
                    channels=128, num_elems=N, d=1, num_idxs=FT)

            def stage_mlp(p):
                t_xg = XG_t.pop(p)
                xi = [_bcast16(t_xtqf[:, 128 * p + 32 * cj:128 * p + 32 * cj + 32], 32)
                      for cj in range(4)]
                # one contiguous r tile: layer blocks ordered r4,r3,r2,r1 to
                # match the output row order, so the aggregation is a single
                # 4D tensor_reduce
                r_all = rp.tile([128, 4 * FT], f32r, tag="rall", name="rall")
                R_t[p] = r_all

                def layer(blk, terms, bias, func):
                    r = r_all[:, blk * FT:(blk + 1) * FT]
                    for cj in range(4):
                        ps = psm.tile([128, 512], f32, tag="ps_mlp", name="ps_mlp")
                        for k, (w, rhs) in enumerate(terms):
                            if rhs is None:
                                rhs_ap = xi[cj]
                            else:
                                rhs_ap = rhs[:, cj * 512:cj * 512 + 512]
                            nc.tensor.matmul(
                                ps[:], t_w[w][:], rhs_ap,
                                start=(k == 0), stop=(k == len(terms) - 1))
                        nc.scalar.activation(
                            r[:, cj * 512:(cj + 1) * 512], ps[:], func,
                            bias=bias[:, 0:1], scale=1.0)
                    return r

                r1 = layer(3, [("w1b", t_xg), ("w1a", None)], t_b[1], relu)
                r2 = layer(2, [("w2r1", r1), ("w2x", None)], t_b[2], relu)
                r3 = layer(1, [("w3r2", r2), ("w3r1", r1), ("w3x", None)],
                           t_b[3], relu)
                layer(0, [("w4r3", r3), ("w4r2", r2), ("w4r1", r1),
                          ("w4x", None)], t_b[4], ident_f)

            def stage_reduce(p):
                t_agg = aggp.tile([128, 512], f32, tag="agg", name="agg")
                AGG_t[p] = t_agg
                r_all = R_t.pop(p)
                for l in range(4):
                    rl = _as_dt(r_all[:, l * FT:(l + 1) * FT], f32)
                    r3d = dataclasses.replace(
                        rl, ap=type(rl.ap)([list(rl.ap[0]), [16, 128], [1, 16]]))
                    nc.vector.tensor_reduce(
                        t_agg[:, 128 * l:128 * l + 128], r3d,
                        axis=mybir.AxisListType.X, op=mx)

            def stage_out(p):
                t_agg = AGG_t.pop(p)
                for l in range(4):
                    nc.gpsimd.dma_start(
                        d_out[64 * l:64 * l + 64, 256 * p:256 * p + 128],
                        t_agg[0:64, 128 * l:128 * l + 128])
                    nc.gpsimd.dma_start(
                        d_out[64 * l:64 * l + 64, 256 * p + 128:256 * p + 256],
                        t_agg[64:128, 128 * l:128 * l + 128])

            # 3-deep software pipeline. Per-engine FIFO per iteration: PE
            # [scores(it), transpose(it-1), mlp(it-2)], ACT [d2 copies(it),
            # idx-cast(it-1), acts(it-2)], DVE [reduce(it-3), sel(it)],
            # gpsimd [gather(it-1), out-dma(it-3)]. The DVE never waits on a
            # same-iteration producer (reduce inputs finished last iteration,
            # sel streams behind the ACT d2 copies), and the gather feeding
            # mlp(p) completes a full iteration before mlp(p) issues.
            for it in range(NPAIR + 4):
                if it < NPAIR:
                    stage_scores(it)
                if 1 <= it <= NPAIR:
                    stage_transpose(it - 1)
                    stage_cidx(it - 1)
                if 4 <= it:
                    stage_reduce(it - 4)
                if 1 <= it <= NPAIR:
                    stage_gather(it - 1)
                if it < NPAIR:
                    stage_sel(it)
                if 4 <= it:
                    stage_out(it - 4)
                if 3 <= it and it - 3 < NPAIR:
                    stage_mlp(it - 3)

    nc.compile()
    return nc


def host_prep(x, pos, W_first, b_first, W_mid1, b_mid1, W_mid2, b_mid2,
              W_last, b_last):
    """Build the 8 per-core input maps (pure marshalling: slicing/stacking)."""
    x = np.ascontiguousarray(np.asarray(x, np.float32))
    pos = np.ascontiguousarray(np.asarray(pos, np.float32))

    def blk(w):
        o = np.zeros((128, 128), np.float32)
        o[:64, :64] = w
        o[64:, 64:] = w
        return o

    Wf = np.asarray(W_first, np.float32)
    A = Wf[0:64] - Wf[128:192]
    Bm = Wf[64:128] + Wf[128:192]
    W1 = np.asarray(W_mid1, np.float32)
    W2 = np.asarray(W_mid2, np.float32)
    W3 = np.asarray(W_last, np.float32)
    weights = {
        "w1b": blk(Bm), "w1a": blk(A),
        "w2r1": blk(W1[0:64]), "w2x": blk(W1[64:128]),
        "w3r2": blk(W2[0:64]), "w3r1": blk(W2[64:128]), "w3x": blk(W2[128:192]),
        "w4r3": blk(W3[0:64]), "w4r2": blk(W3[64:128]), "w4r1": blk(W3[128:192]),
        "w4x": blk(W3[192:256]),
    }
    biases = {f"b{l}": np.ascontiguousarray(
        np.concatenate([bv, bv]).astype(np.float32)[:, None])
        for l, bv in ((1, b_first), (2, b_mid1), (3, b_mid2), (4, b_last))}

    bfnp = ml_dtypes.bfloat16

    def split3(v):
        h = v.astype(bfnp).astype(np.float32)
        m = (v - h).astype(bfnp).astype(np.float32)
        lo = (v - h - m).astype(bfnp).astype(np.float32)
        return h, m, lo

    intc = np.zeros((128, 112), np.uint32)
    intc[:, 0:64] = np.uint32(0xFFFFFFFF) << np.uint32(LB)   # value mask
    intc[:, 64:80] = (1 << LB) - 1                           # position mask
    intc[:, 80:96] = 3                                       # slot >> 3
    intc[:, 96:112] = 9                                      # << log2(CH)

    in_maps = []
    for c in range(8):
        b, h = c // 2, c % 2
        qs = h * NQ
        p = pos[b]
        cn = (p * p).sum(-1).astype(np.float32)
        # bf16 triple-split: K=30 bf16 matmul reproduces -|q-c|^2 to ~6e-6
        Qh, Qm, Ql = split3((2.0 * p).astype(np.float32))   # [N, 3]
        Chs, Cms, Cls = split3(p)
        cnh, cnm, cnl = split3(cn)
        neg1 = -np.ones((3, N), np.float32)
        qaug_f = np.concatenate(
            [Qh.T, Qh.T, Qm.T, Qh.T, Ql.T, Qm.T, Qm.T, Ql.T, neg1,
             np.stack([cnh, cnm, cnl])], 0)
        caug_f = np.concatenate(
            [Chs.T, Cms.T, Chs.T, Cls.T, Chs.T, Cms.T, Cls.T, Cms.T,
             np.stack([cnh, cnm, cnl]), neg1], 0)          # [30, N]
        # roll the candidate order by qs so the self block of query tile t is
        # always at columns [128*t, 128*t+128) regardless of the core's half
        caug = np.ascontiguousarray(np.roll(caug_f, -qs, axis=1).astype(bfnp))
        qaug = np.ascontiguousarray(qaug_f[:, qs:qs + NQ].astype(bfnp))
        xt = np.ascontiguousarray(x[b].T)                   # [64, 4096]
        xtf = np.ascontiguousarray(
            np.roll(np.concatenate([xt, xt], 0), -qs, axis=1))
        xtq = np.ascontiguousarray(xt[:, qs:qs + NQ])
        v = xtq.reshape(64, NPAIR, 2, 128)
        xtqf = np.ascontiguousarray(
            np.concatenate([v[:, :, 0, :], v[:, :, 1, :]], 0).reshape(128, NQ // 2))
        # self-block penalty: the kernel subtracts 2^20 on the diagonal of the
        # 128-col block at qs + 128*t; queries are rows qs.. so the identity
        # works for every tile with the same [128,128] constants
        nbig = np.ascontiguousarray((-np.eye(128) * (2.0 ** 20)).astype(bfnp))
        selfb = np.zeros((128, 2048), np.float32)
        for v in range(4):
            selfb[:, 512 * v + 128 * v:512 * v + 128 * v + 128] = np.eye(128)
        m = dict(caug=caug, qaug=qaug, xtf=xtf, xtqf=xtqf, xtq=xtq,
                 ident=np.eye(128, dtype=np.float32), nbig=nbig,
                 selfb=np.ascontiguousarray(selfb.astype(bfnp)),
                 intc=np.ascontiguousarray(intc), **weights, **biases)
        in_maps.append(m)
    return in_maps


_NC_CACHE = {}


def _get_nc():
    if "nc" not in _NC_CACHE:
        _NC_CACHE["nc"] = build_nc()
    return _NC_CACHE["nc"]


def kernel(**inputs) -> np.ndarray:
    in_maps = host_prep(**inputs)
    nc = _get_nc()
    res = bass_utils.run_bass_kernel_spmd(nc, in_maps, list(range(8)))
    out = np.empty((B, N, OUTF), np.float32)
    for c in range(8):
        b, h = c // 2, c % 2
        out[b, h * NQ:(h + 1) * NQ, :] = res.results[c]["out"].T
    return out


# revision 19
# speedup vs baseline: 1.1375x; 1.1375x over previous
"""DenseEdgeConv (gnn_message_passing) Bass kernel for 8 TRN2 NeuronCores.

Model (B=4, N=4096, D=64, K=16, G=64, L=4):
  knn_idx = 16-NN of pos within each cloud (excluding self)
  edge MLP: 4 dense layers over [x_i, x_j, x_j - x_i] with dense (concat) growth
  out = max over neighbors of [r4, r3, r2, r1, x_i]   -> (B, N, 320)

Sharding: 8 cores = (batch b, query-half h); each core handles 2048 queries of
one cloud with the full cloud replicated (KNN is within-cloud).

Per core: 8 pairs of 128-query tiles, software-pipelined 2 deep so the DVE
selection of pair p overlaps the PE MLP of pair p-2 and gpsimd aggregation of
p-2; engine queues stay dense instead of ping-ponging.

  Selection per tile: PE computes s = 2q.c - |c|^2 - |q|^2 = -|q-c|^2 (+~6e-6)
  with a K=30 bf16 triple-split matmul; a -2^20 identity matmul accumulated on
  the self block removes self. ACT copies PSUM->SBUF. DVE takes top-8 per
  512-chunk (max8), recovers within-chunk positions (max_index), packs the
  9-bit position into the low mantissa bits of the score (monotone for the
  all-negative scores; costs 2^-14 relative quantization, validated 23/32768
  wrong neighbor sets on the actual data), merges to the top-16 with
  max8+match_replace on the packed values, and recovers each winner's chunk
  with a 64-wide max_index. Global ids are rebuilt with two ALU ops - no
  full-row index scan.

  MLP per pair: neighbor ids are PE-transposed into gpsimd's 16-wrapped
  layout, ap_gather pulls neighbor feature columns, and blockdiag-packed f32r
  matmuls (two 512-token folds per instruction) run the 4 layers with
  per-point terms via step-0 broadcast APs; ACT applies bias+relu from PSUM.
  The 16-neighbor max aggregation runs as a 4-round tournament on gpsimd
  (DVE is the critical engine; gpsimd has slack).
"""

import contextlib
import dataclasses

import ml_dtypes
import numpy as np

import concourse.bacc as bacc
import concourse.mybir as mybir
import concourse.tile as tile
from concourse import bass_utils

B, N, D, K16, G = 4, 4096, 64, 16, 64
NQ = N // 2            # queries per core
NTILE = NQ // 128      # 16 query tiles per core
NPAIR = NTILE // 2     # 8 tile pairs
FT = 256 * K16 // 2    # 2048 folded columns per pair (4096 tokens)
CH = 512               # selection chunk size
NCH = N // CH          # 8 chunks
LB = 9                 # position bits packed into score mantissa
OUTF = D + 4 * G       # 320 output features
KAUG = 30              # bf16 triple-split score lanes

f32 = mybir.dt.float32
f32r = mybir.dt.float32r
bf16 = mybir.dt.bfloat16
u32 = mybir.dt.uint32
i16 = mybir.dt.int16


def _as_dt(ap, dt):
    t = dataclasses.replace(ap.tensor, dtype=dt)
    return dataclasses.replace(ap, tensor=t)


def _stride2(ap, n, off):
    # view [p, 2n] as [p, n] with step 2, starting at element `off`
    return dataclasses.replace(
        ap, offset=ap.offset + off, ap=type(ap.ap)([list(ap.ap[0]), [2, n]])
    )


def _bcast16(ap, cols):
    # [p, cols] slice -> [p, cols, 16] with step-0 inner dim (16x per-query repeat)
    return dataclasses.replace(
        ap, ap=type(ap.ap)([list(ap.ap[0]), [1, cols], [0, 16]])
    )


def _out3d(ap):
    # d_out[0:256, c0:c0+128] -> [64 rows, 4 layer-blocks, 128 cols] view
    return dataclasses.replace(
        ap, ap=type(ap.ap)([[N // 2, 64], [64 * (N // 2), 4], [1, 128]])
    )


def _r4d(ap):
    # [128, 4*FT] r tile -> [128, 4 layers, 128 queries, 16 neighbors]
    return dataclasses.replace(
        ap, ap=type(ap.ap)([list(ap.ap[0]), [FT, 4], [16, 128], [1, 16]])
    )


def build_nc():
    nc = bacc.Bacc(None, target_bir_lowering=False)

    d_caug = nc.dram_tensor("caug", [KAUG, N], bf16, kind="ExternalInput")
    d_qaug = nc.dram_tensor("qaug", [KAUG, NQ], bf16, kind="ExternalInput")
    d_xtf = nc.dram_tensor("xtf", [128, N], f32, kind="ExternalInput")
    d_xtqf = nc.dram_tensor("xtqf", [128, NQ // 2], f32r, kind="ExternalInput")
    d_xtq = nc.dram_tensor("xtq", [D, NQ], f32, kind="ExternalInput")
    WNAMES = ["w1b", "w1a", "w2r1", "w2x", "w3r2", "w3r1", "w3x",
              "w4r3", "w4r2", "w4r1", "w4x"]
    d_w = {n: nc.dram_tensor(n, [128, 128], f32 if n == "w1b" else f32r,
                             kind="ExternalInput") for n in WNAMES}
    d_b = {l: nc.dram_tensor(f"b{l}", [128, 1], f32, kind="ExternalInput")
           for l in (1, 2, 3, 4)}
    d_ident = nc.dram_tensor("ident", [128, 128], f32, kind="ExternalInput")
    d_nbig = nc.dram_tensor("nbig", [128, 128], bf16, kind="ExternalInput")
    d_selfb = nc.dram_tensor("selfb", [128, 2048], bf16, kind="ExternalInput")
    d_ic = nc.dram_tensor("intc", [128, 112], u32, kind="ExternalInput")
    d_out = nc.dram_tensor("out", [OUTF, NQ], f32, kind="ExternalOutput")

    with tile.TileContext(nc) as tc:
        ctx = contextlib.ExitStack()
        with ctx:
            const = ctx.enter_context(tc.tile_pool(name="const", bufs=1))
            t_caug = const.tile([KAUG, N], bf16)
            t_qaug = const.tile([KAUG, NQ], bf16)
            t_xtf = const.tile([128, N], f32)
            t_xtqf = const.tile([128, NQ // 2], f32r)
            t_w = {n: const.tile([128, 128], f32 if n == "w1b" else f32r,
                                 tag=f"w_{n}", name=f"w_{n}") for n in WNAMES}
            t_b = {l: const.tile([128, 1], f32, tag=f"b_{l}", name=f"b_{l}")
                   for l in (1, 2, 3, 4)}
            t_ident = const.tile([128, 128], f32)
            t_nbig = const.tile([128, 128], bf16)
            t_selfb = const.tile([128, 2048], bf16)
            t_ic = const.tile([128, 112], u32)
            for dst, src in ((t_caug, d_caug), (t_qaug, d_qaug),
                             (t_xtf, d_xtf), (t_xtqf, d_xtqf),
                             (t_ident, d_ident), (t_nbig, d_nbig),
                             (t_selfb, d_selfb), (t_ic, d_ic)):
                nc.sync.dma_start(dst[:], src[:])
            for n in WNAMES:
                nc.sync.dma_start(t_w[n][:], d_w[n][:])
            for l in (1, 2, 3, 4):
                nc.sync.dma_start(t_b[l][:], d_b[l][:])
            # x_i part of the output passes straight through
            nc.sync.dma_start(d_out[4 * G:OUTF, :], d_xtq[:])

            psd = ctx.enter_context(tc.tile_pool(name="psd", bufs=2, space="PSUM"))
            psm = ctx.enter_context(tc.tile_pool(name="psm", bufs=2, space="PSUM"))
            pst = ctx.enter_context(tc.tile_pool(name="pst", bufs=2, space="PSUM"))
            d2p = ctx.enter_context(tc.tile_pool(name="d2p", bufs=4))
            selp = ctx.enter_context(tc.tile_pool(name="selp", bufs=2))
            sp = ctx.enter_context(tc.tile_pool(name="sp", bufs=2))
            idxp = ctx.enter_context(tc.tile_pool(name="idxp", bufs=2))
            xgp = ctx.enter_context(tc.tile_pool(name="xgp", bufs=3))
            rp = ctx.enter_context(tc.tile_pool(name="rp", bufs=2))
            aggp = ctx.enter_context(tc.tile_pool(name="aggp", bufs=2))

            relu = mybir.ActivationFunctionType.Relu
            ident_f = mybir.ActivationFunctionType.Identity
            mx = mybir.AluOpType.max
            alu = mybir.AluOpType

            S_t, IDX_t, XG_t, R_t, AGG_t = {}, {}, {}, {}, {}

            D2_t = {}

            def stage_scores(p):
                for sub in range(2):
                    t = 2 * p + sub
                    # host rolls the candidate order by each core's query
                    # offset, so tile t's self block is always at 128*t
                    c_self = 128 * t
                    t_d2 = d2p.tile([128, N], f32, tag="d2sb", name="d2sb")
                    D2_t[t] = t_d2
                    for quarter in range(4):
                        p_d2 = psd.tile([128, 1024], f32, tag="psd2", name="psd2")
                        c0 = quarter * 1024
                        for j in range(2):
                            lo = c0 + j * 512
                            has_self = lo <= c_self < lo + 512
                            nc.tensor.matmul(
                                p_d2[:, j * 512:(j + 1) * 512],
                                t_qaug[:, t * 128:(t + 1) * 128],
                                t_caug[:, lo:lo + 512],
                                start=True, stop=not has_self)
                            if has_self:
                                v = t % 4    # identity offset within the half
                                nc.tensor.matmul(
                                    p_d2[:, j * 512:(j + 1) * 512],
                                    t_nbig[:],
                                    t_selfb[:, 512 * v:512 * v + 512],
                                    start=False, stop=True)
                        nc.scalar.copy(t_d2[:, c0:c0 + 1024], p_d2[:])

            def stage_sel(p):
                t_S = sp.tile([128, 128], f32, tag="S", name="S")
                S_t[p] = t_S
                for sub in range(2):
                    t = 2 * p + sub
                    t_d2 = D2_t.pop(t)
                    # L1: top-8 values + within-chunk positions per 512-chunk
                    t_V = selp.tile([128, 8 * NCH], f32, tag="V", name="V")
                    t_I = selp.tile([128, 8 * NCH], u32, tag="I", name="I")
                    for c in range(NCH):
                        nc.vector.max(t_V[:, 8 * c:8 * c + 8],
                                      t_d2[:, CH * c:CH * (c + 1)])
                    for c in range(NCH):
                        nc.vector.max_index(t_I[:, 8 * c:8 * c + 8],
                                            t_V[:, 8 * c:8 * c + 8],
                                            t_d2[:, CH * c:CH * (c + 1)])
                    # pack positions into the low LB mantissa bits
                    t_P = selp.tile([128, 8 * NCH], f32, tag="P", name="P")
                    nc.vector.tensor_tensor(
                        _as_dt(t_P[:], u32), _as_dt(t_V[:], u32),
                        t_ic[:, 0:64], op=alu.bitwise_and)
                    nc.vector.tensor_tensor(
                        _as_dt(t_P[:], u32), _as_dt(t_P[:], u32), t_I[:],
                        op=alu.bitwise_or)
                    # merge to top-16 packed + candidate slots
                    t_v16 = selp.tile([128, 16], f32, tag="v16", name="v16")
                    t_pos = selp.tile([128, 16], u32, tag="pos", name="pos")
                    nc.vector.max(t_v16[:, 0:8], t_P[:])
                    nc.vector.max_index(t_pos[:, 0:8], t_v16[:, 0:8], t_P[:])
                    nc.vector.match_replace(
                        t_P[:], in_to_replace=t_v16[:, 0:8], in_values=t_P[:],
                        imm_value=-1e30)
                    nc.vector.max(t_v16[:, 8:16], t_P[:])
                    nc.vector.max_index(t_pos[:, 8:16], t_v16[:, 8:16], t_P[:])
                    # global id = (slot>>3)*CH + (packed & (2^LB-1))
                    t_cb = selp.tile([128, 16], u32, tag="cb", name="cb")
                    nc.vector.tensor_tensor(
                        t_cb[:], t_pos[:], t_ic[:, 80:96],
                        op=alu.logical_shift_right)
                    nc.vector.tensor_tensor(
                        t_cb[:], t_cb[:], t_ic[:, 96:112],
                        op=alu.logical_shift_left)
                    t_g = selp.tile([128, 16], u32, tag="g", name="g")
                    nc.vector.tensor_tensor(
                        t_g[:], _as_dt(t_v16[:], u32), t_ic[:, 64:80],
                        op=alu.bitwise_and)
                    nc.vector.tensor_tensor(
                        t_g[:], t_g[:], t_cb[:], op=alu.add)
                    # write ids into S (as f32) and replicate 3x for the
                    # per-16-partition gather wrap
                    s_blk = t_S[:, 64 * sub:64 * sub + 16]
                    nc.vector.tensor_copy(s_blk, t_g[:])
                    rep_out = dataclasses.replace(
                        t_S[:, 64 * sub + 16:64 * sub + 64],
                        ap=type(s_blk.ap)([list(s_blk.ap[0]), [16, 3], [1, 16]]))
                    rep_in = dataclasses.replace(
                        s_blk, ap=type(s_blk.ap)([list(s_blk.ap[0]), [0, 3], [1, 16]]))
                    nc.vector.tensor_copy(rep_out, rep_in)

            PT_t = {}

            def stage_transpose(p):
                p_T = pst.tile([128, 128], f32, tag="ptr", name="ptr")
                PT_t[p] = p_T
                nc.tensor.transpose(p_T[:], S_t.pop(p)[:], t_ident[:])

            def stage_cidx(p):
                # PSUM->SBUF i16 cast on the ACT queue: keeps the DVE free of
                # any same-iteration PE dependency
                t_IDX = idxp.tile([128, 128], i16, tag="IDX", name="IDX")
                IDX_t[p] = t_IDX
                nc.scalar.copy(t_IDX[:], PT_t.pop(p)[:])

            def stage_gather(p):
                t_xg = xgp.tile([128, FT], f32, tag="xg", name="xg")
                XG_t[p] = t_xg
                nc.gpsimd.ap_gather(
                    t_xg[:].rearrange("c (n d) -> c n d", d=1),
                    t_xtf[:].rearrange("c (n d) -> c n d", d=1),
                    IDX_t.pop(p)[:],
                    channels=128, num_elems=N, d=1, num_idxs=FT)

            def stage_mlp(p):
                t_xg = XG_t.pop(p)
                xi = [_bcast16(t_xtqf[:, 128 * p + 32 * cj:128 * p + 32 * cj + 32], 32)
                      for cj in range(4)]
                # one contiguous r tile: layer blocks ordered r4,r3,r2,r1 to
                # match the output row order, so the aggregation is a single
                # 4D tensor_reduce
                r_all = rp.tile([128, 4 * FT], f32r, tag="rall", name="rall")
                R_t[p] = r_all

                def layer(blk, terms, bias, func):
                    r = r_all[:, blk * FT:(blk + 1) * FT]
                    for cj in range(4):
                        ps = psm.tile([128, 512], f32, tag="ps_mlp", name="ps_mlp")
                        for k, (w, rhs) in enumerate(terms):
                            if rhs is None:
                                rhs_ap = xi[cj]
                            else:
                                rhs_ap = rhs[:, cj * 512:cj * 512 + 512]
                            nc.tensor.matmul(
                                ps[:], t_w[w][:], rhs_ap,
                                start=(k == 0), stop=(k == len(terms) - 1))
                        nc.scalar.activation(
                            r[:, cj * 512:(cj + 1) * 512], ps[:], func,
                            bias=bias[:, 0:1], scale=1.0)
                    return r

                r1 = layer(3, [("w1b", t_xg), ("w1a", None)], t_b[1], relu)
                r2 = layer(2, [("w2r1", r1), ("w2x", None)], t_b[2], relu)
                r3 = layer(1, [("w3r2", r2), ("w3r1", r1), ("w3x", None)],
                           t_b[3], relu)
                layer(0, [("w4r3", r3), ("w4r2", r2), ("w4r1", r1),
                          ("w4x", None)], t_b[4], ident_f)

            def stage_reduce(p):
                t_agg = aggp.tile([128, 512], f32, tag="agg", name="agg")
                AGG_t[p] = t_agg
                r_all = R_t.pop(p)
                for l in range(4):
                    rl = _as_dt(r_all[:, l * FT:(l + 1) * FT], f32)
                    r3d = dataclasses.replace(
                        rl, ap=type(rl.ap)([list(rl.ap[0]), [16, 128], [1, 16]]))
                    nc.vector.tensor_reduce(
                        t_agg[:, 128 * l:128 * l + 128], r3d,
                        axis=mybir.AxisListType.X, op=mx)

            def stage_out(p):
                t_agg = AGG_t.pop(p)
                for l in range(4):
                    nc.gpsimd.dma_start(
                        d_out[64 * l:64 * l + 64, 256 * p:256 * p + 128],
                        t_agg[0:64, 128 * l:128 * l + 128])
                    nc.gpsimd.dma_start(
                        d_out[64 * l:64 * l + 64, 256 * p + 128:256 * p + 256],
                        t_agg[64:128, 128 * l:128 * l + 128])

            # 3-deep software pipeline. Per-engine FIFO per iteration: PE
            # [scores(it), transpose(it-1), mlp(it-2)], ACT [d2 copies(it),
            # idx-cast(it-1), acts(it-2)], DVE [reduce(it-3), sel(it)],
            # gpsimd [gather(it-1), out-dma(it-3)]. The DVE never waits on a
            # same-iteration producer (reduce inputs finished last iteration,
            # sel streams behind the ACT d2 copies), and the gather feeding
            # mlp(p) completes a full iteration before mlp(p) issues.
            for it in range(NPAIR + 3):
                if it < NPAIR:
                    stage_scores(it)
                if 1 <= it <= NPAIR:
                    # keep the id-wrap path ahead of the mlp in the
                    # scheduler's priority heap: a late transpose/gather
                    # serializes every downstream engine
                    with tc.high_priority():
                        stage_transpose(it - 1)
                        stage_cidx(it - 1)
                        stage_gather(it - 1)
                if 3 <= it:
                    stage_reduce(it - 3)
                if it < NPAIR:
                    stage_sel(it)
                if 3 <= it:
                    stage_out(it - 3)
                if 2 <= it and it - 2 < NPAIR:
                    stage_mlp(it - 2)

    nc.compile()
    return nc


def host_prep(x, pos, W_first, b_first, W_mid1, b_mid1, W_mid2, b_mid2,
              W_last, b_last):
    """Build the 8 per-core input maps (pure marshalling: slicing/stacking)."""
    x = np.ascontiguousarray(np.asarray(x, np.float32))
    pos = np.ascontiguousarray(np.asarray(pos, np.float32))

    def blk(w):
        o = np.zeros((128, 128), np.float32)
        o[:64, :64] = w
        o[64:, 64:] = w
        return o

    Wf = np.asarray(W_first, np.float32)
    A = Wf[0:64] - Wf[128:192]
    Bm = Wf[64:128] + Wf[128:192]
    W1 = np.asarray(W_mid1, np.float32)
    W2 = np.asarray(W_mid2, np.float32)
    W3 = np.asarray(W_last, np.float32)
    weights = {
        "w1b": blk(Bm), "w1a": blk(A),
        "w2r1": blk(W1[0:64]), "w2x": blk(W1[64:128]),
        "w3r2": blk(W2[0:64]), "w3r1": blk(W2[64:128]), "w3x": blk(W2[128:192]),
        "w4r3": blk(W3[0:64]), "w4r2": blk(W3[64:128]), "w4r1": blk(W3[128:192]),
        "w4x": blk(W3[192:256]),
    }
    biases = {f"b{l}": np.ascontiguousarray(
        np.concatenate([bv, bv]).astype(np.float32)[:, None])
        for l, bv in ((1, b_first), (2, b_mid1), (3, b_mid2), (4, b_last))}

    bfnp = ml_dtypes.bfloat16

    def split3(v):
        h = v.astype(bfnp).astype(np.float32)
        m = (v - h).astype(bfnp).astype(np.float32)
        lo = (v - h - m).astype(bfnp).astype(np.float32)
        return h, m, lo

    intc = np.zeros((128, 112), np.uint32)
    intc[:, 0:64] = np.uint32(0xFFFFFFFF) << np.uint32(LB)   # value mask
    intc[:, 64:80] = (1 << LB) - 1                           # position mask
    intc[:, 80:96] = 3                                       # slot >> 3
    intc[:, 96:112] = 9                                      # << log2(CH)

    in_maps = []
    for c in range(8):
        b, h = c // 2, c % 2
        qs = h * NQ
        p = pos[b]
        cn = (p * p).sum(-1).astype(np.float32)
        # bf16 triple-split: K=30 bf16 matmul reproduces -|q-c|^2 to ~6e-6
        Qh, Qm, Ql = split3((2.0 * p).astype(np.float32))   # [N, 3]
        Chs, Cms, Cls = split3(p)
        cnh, cnm, cnl = split3(cn)
        neg1 = -np.ones((3, N), np.float32)
        qaug_f = np.concatenate(
            [Qh.T, Qh.T, Qm.T, Qh.T, Ql.T, Qm.T, Qm.T, Ql.T, neg1,
             np.stack([cnh, cnm, cnl])], 0)
        caug_f = np.concatenate(
            [Chs.T, Cms.T, Chs.T, Cls.T, Chs.T, Cms.T, Cls.T, Cms.T,
             np.stack([cnh, cnm, cnl]), neg1], 0)          # [30, N]
        # roll the candidate order by qs so the self block of query tile t is
        # always at columns [128*t, 128*t+128) regardless of the core's half
        caug = np.ascontiguousarray(np.roll(caug_f, -qs, axis=1).astype(bfnp))
        qaug = np.ascontiguousarray(qaug_f[:, qs:qs + NQ].astype(bfnp))
        xt = np.ascontiguousarray(x[b].T)                   # [64, 4096]
        xtf = np.ascontiguousarray(
            np.roll(np.concatenate([xt, xt], 0), -qs, axis=1))
        xtq = np.ascontiguousarray(xt[:, qs:qs + NQ])
        v = xtq.reshape(64, NPAIR, 2, 128)
        xtqf = np.ascontiguousarray(
            np.concatenate([v[:, :, 0, :], v[:, :, 1, :]], 0).reshape(128, NQ // 2))
        # self-block penalty: the kernel subtracts 2^20 on the diagonal of the
        # 128-col block at qs + 128*t; queries are rows qs.. so the identity
        # works for every tile with the same [128,128] constants
        nbig = np.ascontiguousarray((-np.eye(128) * (2.0 ** 20)).astype(bfnp))
        selfb = np.zeros((128, 2048), np.float32)
        for v in range(4):
            selfb[:, 512 * v + 128 * v:512 * v + 128 * v + 128] = np.eye(128)
        m = dict(caug=caug, qaug=qaug, xtf=xtf, xtqf=xtqf, xtq=xtq,
                 ident=np.eye(128, dtype=np.float32), nbig=nbig,
                 selfb=np.ascontiguousarray(selfb.astype(bfnp)),
                 intc=np.ascontiguousarray(intc), **weights, **biases)
        in_maps.append(m)
    return in_maps


_NC_CACHE = {}


def _get_nc():
    if "nc" not in _NC_CACHE:
        _NC_CACHE["nc"] = build_nc()
    return _NC_CACHE["nc"]


def kernel(**inputs) -> np.ndarray:
    in_maps = host_prep(**inputs)
    nc = _get_nc()
    res = bass_utils.run_bass_kernel_spmd(nc, in_maps, list(range(8)))
    out = np.empty((B, N, OUTF), np.float32)
    for c in range(8):
        b, h = c // 2, c % 2
        out[b, h * NQ:(h + 1) * NQ, :] = res.results[c]["out"].T
    return out


# revision 22
# speedup vs baseline: 1.1380x; 1.0004x over previous
"""DenseEdgeConv (gnn_message_passing) Bass kernel for 8 TRN2 NeuronCores.

Model (B=4, N=4096, D=64, K=16, G=64, L=4):
  knn_idx = 16-NN of pos within each cloud (excluding self)
  edge MLP: 4 dense layers over [x_i, x_j, x_j - x_i] with dense (concat) growth
  out = max over neighbors of [r4, r3, r2, r1, x_i]   -> (B, N, 320)

Sharding: 8 cores = (batch b, query-half h); each core handles 2048 queries of
one cloud with the full cloud replicated (KNN is within-cloud).

Per core: 8 pairs of 128-query tiles, software-pipelined 2 deep so the DVE
selection of pair p overlaps the PE MLP of pair p-2 and gpsimd aggregation of
p-2; engine queues stay dense instead of ping-ponging.

  Selection per tile: PE computes s = 2q.c - |c|^2 - |q|^2 = -|q-c|^2 (+~6e-6)
  with a K=30 bf16 triple-split matmul; a -2^20 identity matmul accumulated on
  the self block removes self. ACT copies PSUM->SBUF. DVE takes top-8 per
  512-chunk (max8), recovers within-chunk positions (max_index), packs the
  9-bit position into the low mantissa bits of the score (monotone for the
  all-negative scores; costs 2^-14 relative quantization, validated 23/32768
  wrong neighbor sets on the actual data), merges to the top-16 with
  max8+match_replace on the packed values, and recovers each winner's chunk
  with a 64-wide max_index. Global ids are rebuilt with two ALU ops - no
  full-row index scan.

  MLP per pair: neighbor ids are PE-transposed into gpsimd's 16-wrapped
  layout, ap_gather pulls neighbor feature columns, and blockdiag-packed f32r
  matmuls (two 512-token folds per instruction) run the 4 layers with
  per-point terms via step-0 broadcast APs; ACT applies bias+relu from PSUM.
  The 16-neighbor max aggregation runs as a 4-round tournament on gpsimd
  (DVE is the critical engine; gpsimd has slack).
"""

import contextlib
import dataclasses

import ml_dtypes
import numpy as np

import concourse.bacc as bacc
import concourse.mybir as mybir
import concourse.tile as tile
from concourse import bass_utils

B, N, D, K16, G = 4, 4096, 64, 16, 64
NQ = N // 2            # queries per core
NTILE = NQ // 128      # 16 query tiles per core
NPAIR = NTILE // 2     # 8 tile pairs
FT = 256 * K16 // 2    # 2048 folded columns per pair (4096 tokens)
CH = 512               # selection chunk size
NCH = N // CH          # 8 chunks
LB = 9                 # position bits packed into score mantissa
OUTF = D + 4 * G       # 320 output features
KAUG = 30              # bf16 triple-split score lanes

f32 = mybir.dt.float32
f32r = mybir.dt.float32r
bf16 = mybir.dt.bfloat16
u32 = mybir.dt.uint32
i16 = mybir.dt.int16


def _as_dt(ap, dt):
    t = dataclasses.replace(ap.tensor, dtype=dt)
    return dataclasses.replace(ap, tensor=t)


def _stride2(ap, n, off):
    # view [p, 2n] as [p, n] with step 2, starting at element `off`
    return dataclasses.replace(
        ap, offset=ap.offset + off, ap=type(ap.ap)([list(ap.ap[0]), [2, n]])
    )


def _bcast16(ap, cols):
    # [p, cols] slice -> [p, cols, 16] with step-0 inner dim (16x per-query repeat)
    return dataclasses.replace(
        ap, ap=type(ap.ap)([list(ap.ap[0]), [1, cols], [0, 16]])
    )


def _out3d(ap):
    # d_out[0:256, c0:c0+128] -> [64 rows, 4 layer-blocks, 128 cols] view
    return dataclasses.replace(
        ap, ap=type(ap.ap)([[N // 2, 64], [64 * (N // 2), 4], [1, 128]])
    )


def _r4d(ap):
    # [128, 4*FT] r tile -> [128, 4 layers, 128 queries, 16 neighbors]
    return dataclasses.replace(
        ap, ap=type(ap.ap)([list(ap.ap[0]), [FT, 4], [16, 128], [1, 16]])
    )


def build_nc():
    nc = bacc.Bacc(None, target_bir_lowering=False)

    d_caug = nc.dram_tensor("caug", [KAUG, N], bf16, kind="ExternalInput")
    d_qaug = nc.dram_tensor("qaug", [KAUG, NQ], bf16, kind="ExternalInput")
    d_xtf = nc.dram_tensor("xtf", [128, N], f32, kind="ExternalInput")
    d_xtqf = nc.dram_tensor("xtqf", [128, NQ // 2], f32r, kind="ExternalInput")
    d_xtq = nc.dram_tensor("xtq", [D, NQ], f32, kind="ExternalInput")
    WNAMES = ["w1b", "w1a", "w2r1", "w2x", "w3r2", "w3r1", "w3x",
              "w4r3", "w4r2", "w4r1", "w4x"]
    RW = {"w2r1", "w3r2", "w3r1", "w4r3", "w4r2", "w4r1"}
    def _wdt(n):
        return f32 if n == "w1b" else (bf16 if n in RW else f32r)
    d_w = {n: nc.dram_tensor(n, [128, 128], _wdt(n), kind="ExternalInput")
           for n in WNAMES}
    d_b = {l: nc.dram_tensor(f"b{l}", [128, 1], f32, kind="ExternalInput")
           for l in (1, 2, 3, 4)}
    d_ident = nc.dram_tensor("ident", [128, 128], f32, kind="ExternalInput")
    d_nbig = nc.dram_tensor("nbig", [128, 128], bf16, kind="ExternalInput")
    d_selfb = nc.dram_tensor("selfb", [128, 2048], bf16, kind="ExternalInput")
    d_ic = nc.dram_tensor("intc", [128, 112], u32, kind="ExternalInput")
    d_out = nc.dram_tensor("out", [OUTF, NQ], f32, kind="ExternalOutput")

    with tile.TileContext(nc) as tc:
        ctx = contextlib.ExitStack()
        with ctx:
            const = ctx.enter_context(tc.tile_pool(name="const", bufs=1))
            t_caug = const.tile([KAUG, N], bf16)
            t_qaug = const.tile([KAUG, NQ], bf16)
            t_xtf = const.tile([128, N], f32)
            t_xtqf = const.tile([128, NQ // 2], f32r)
            t_w = {n: const.tile([128, 128], _wdt(n), tag=f"w_{n}",
                                 name=f"w_{n}") for n in WNAMES}
            t_b = {l: const.tile([128, 1], f32, tag=f"b_{l}", name=f"b_{l}")
                   for l in (1, 2, 3, 4)}
            t_ident = const.tile([128, 128], f32)
            t_nbig = const.tile([128, 128], bf16)
            t_selfb = const.tile([128, 2048], bf16)
            t_ic = const.tile([128, 112], u32)
            for dst, src in ((t_caug, d_caug), (t_qaug, d_qaug),
                             (t_xtf, d_xtf), (t_xtqf, d_xtqf),
                             (t_ident, d_ident), (t_nbig, d_nbig),
                             (t_selfb, d_selfb), (t_ic, d_ic)):
                nc.sync.dma_start(dst[:], src[:])
            for n in WNAMES:
                nc.sync.dma_start(t_w[n][:], d_w[n][:])
            for l in (1, 2, 3, 4):
                nc.sync.dma_start(t_b[l][:], d_b[l][:])
            # x_i part of the output passes straight through
            nc.sync.dma_start(d_out[4 * G:OUTF, :], d_xtq[:])

            psd = ctx.enter_context(tc.tile_pool(name="psd", bufs=2, space="PSUM"))
            psm = ctx.enter_context(tc.tile_pool(name="psm", bufs=2, space="PSUM"))
            pst = ctx.enter_context(tc.tile_pool(name="pst", bufs=2, space="PSUM"))
            d2p = ctx.enter_context(tc.tile_pool(name="d2p", bufs=6))
            selp = ctx.enter_context(tc.tile_pool(name="selp", bufs=2))
            sp = ctx.enter_context(tc.tile_pool(name="sp", bufs=2))
            idxp = ctx.enter_context(tc.tile_pool(name="idxp", bufs=2))
            xgp = ctx.enter_context(tc.tile_pool(name="xgp", bufs=3))
            rp = ctx.enter_context(tc.tile_pool(name="rp", bufs=2))
            aggp = ctx.enter_context(tc.tile_pool(name="aggp", bufs=2))

            relu = mybir.ActivationFunctionType.Relu
            ident_f = mybir.ActivationFunctionType.Identity
            mx = mybir.AluOpType.max
            alu = mybir.AluOpType

            S_t, IDX_t, XG_t, R_t, AGG_t = {}, {}, {}, {}, {}

            D2_t = {}

            def stage_scores(p):
                for sub in range(2):
                    t = 2 * p + sub
                    # host rolls the candidate order by each core's query
                    # offset, so tile t's self block is always at 128*t
                    c_self = 128 * t
                    t_d2 = d2p.tile([128, N], f32, tag="d2sb", name="d2sb")
                    D2_t[t] = t_d2
                    for quarter in range(4):
                        p_d2 = psd.tile([128, 1024], f32, tag="psd2", name="psd2")
                        c0 = quarter * 1024
                        for j in range(2):
                            lo = c0 + j * 512
                            has_self = lo <= c_self < lo + 512
                            nc.tensor.matmul(
                                p_d2[:, j * 512:(j + 1) * 512],
                                t_qaug[:, t * 128:(t + 1) * 128],
                                t_caug[:, lo:lo + 512],
                                start=True, stop=not has_self)
                            if has_self:
                                v = t % 4    # identity offset within the half
                                nc.tensor.matmul(
                                    p_d2[:, j * 512:(j + 1) * 512],
                                    t_nbig[:],
                                    t_selfb[:, 512 * v:512 * v + 512],
                                    start=False, stop=True)
                        nc.scalar.copy(t_d2[:, c0:c0 + 1024], p_d2[:])

            def stage_sel(p):
                t_S = sp.tile([128, 128], f32, tag="S", name="S")
                S_t[p] = t_S
                for sub in range(2):
                    t = 2 * p + sub
                    t_d2 = D2_t.pop(t)
                    # L1: top-8 values + within-chunk positions per 512-chunk
                    t_V = selp.tile([128, 8 * NCH], f32, tag="V", name="V")
                    t_I = selp.tile([128, 8 * NCH], u32, tag="I", name="I")
                    for c in range(NCH):
                        nc.vector.max(t_V[:, 8 * c:8 * c + 8],
                                      t_d2[:, CH * c:CH * (c + 1)])
                    for c in range(NCH):
                        nc.vector.max_index(t_I[:, 8 * c:8 * c + 8],
                                            t_V[:, 8 * c:8 * c + 8],
                                            t_d2[:, CH * c:CH * (c + 1)])
                    # pack positions into the low LB mantissa bits
                    t_P = selp.tile([128, 8 * NCH], f32, tag="P", name="P")
                    nc.vector.tensor_tensor(
                        _as_dt(t_P[:], u32), _as_dt(t_V[:], u32),
                        t_ic[:, 0:64], op=alu.bitwise_and)
                    nc.vector.tensor_tensor(
                        _as_dt(t_P[:], u32), _as_dt(t_P[:], u32), t_I[:],
                        op=alu.bitwise_or)
                    # merge to top-16 packed + candidate slots
                    t_v16 = selp.tile([128, 16], f32, tag="v16", name="v16")
                    t_pos = selp.tile([128, 16], u32, tag="pos", name="pos")
                    nc.vector.max(t_v16[:, 0:8], t_P[:])
                    nc.vector.max_index(t_pos[:, 0:8], t_v16[:, 0:8], t_P[:])
                    nc.vector.match_replace(
                        t_P[:], in_to_replace=t_v16[:, 0:8], in_values=t_P[:],
                        imm_value=-1e30)
                    nc.vector.max(t_v16[:, 8:16], t_P[:])
                    nc.vector.max_index(t_pos[:, 8:16], t_v16[:, 8:16], t_P[:])
                    # global id = (slot>>3)*CH + (packed & (2^LB-1))
                    t_cb = selp.tile([128, 16], u32, tag="cb", name="cb")
                    nc.vector.tensor_tensor(
                        t_cb[:], t_pos[:], t_ic[:, 80:96],
                        op=alu.logical_shift_right)
                    nc.vector.tensor_tensor(
                        t_cb[:], t_cb[:], t_ic[:, 96:112],
                        op=alu.logical_shift_left)
                    t_g = selp.tile([128, 16], u32, tag="g", name="g")
                    nc.vector.tensor_tensor(
                        t_g[:], _as_dt(t_v16[:], u32), t_ic[:, 64:80],
                        op=alu.bitwise_and)
                    nc.vector.tensor_tensor(
                        t_g[:], t_g[:], t_cb[:], op=alu.add)
                    # write ids into S (as f32) and replicate 3x for the
                    # per-16-partition gather wrap
                    s_blk = t_S[:, 64 * sub:64 * sub + 16]
                    nc.vector.tensor_copy(s_blk, t_g[:])
                    rep_out = dataclasses.replace(
                        t_S[:, 64 * sub + 16:64 * sub + 64],
                        ap=type(s_blk.ap)([list(s_blk.ap[0]), [16, 3], [1, 16]]))
                    rep_in = dataclasses.replace(
                        s_blk, ap=type(s_blk.ap)([list(s_blk.ap[0]), [0, 3], [1, 16]]))
                    nc.vector.tensor_copy(rep_out, rep_in)

            PT_t = {}

            def stage_transpose(p):
                p_T = pst.tile([128, 128], f32, tag="ptr", name="ptr")
                PT_t[p] = p_T
                nc.tensor.transpose(p_T[:], S_t.pop(p)[:], t_ident[:])

            def stage_cidx(p):
                # PSUM->SBUF i16 cast on the ACT queue: keeps the DVE free of
                # any same-iteration PE dependency
                t_IDX = idxp.tile([128, 128], i16, tag="IDX", name="IDX")
                IDX_t[p] = t_IDX
                nc.scalar.copy(t_IDX[:], PT_t.pop(p)[:])

            def stage_gather(p):
                t_xg = xgp.tile([128, FT], f32, tag="xg", name="xg")
                XG_t[p] = t_xg
                nc.gpsimd.ap_gather(
                    t_xg[:].rearrange("c (n d) -> c n d", d=1),
                    t_xtf[:].rearrange("c (n d) -> c n d", d=1),
                    IDX_t.pop(p)[:],
                    channels=128, num_elems=N, d=1, num_idxs=FT)

            def stage_mlp(p):
                t_xg = XG_t.pop(p)
                xi = [_bcast16(t_xtqf[:, 128 * p + 32 * cj:128 * p + 32 * cj + 32], 32)
                      for cj in range(4)]
                # one contiguous r tile: layer blocks ordered r4,r3,r2,r1 to
                # match the output row order, so the aggregation is a single
                # 4D tensor_reduce
                r_all = rp.tile([128, 4 * FT], bf16, tag="rall", name="rall")
                R_t[p] = r_all

                def layer(blk, terms, bias, func):
                    r = r_all[:, blk * FT:(blk + 1) * FT]
                    for cj in range(4):
                        ps = psm.tile([128, 512], f32, tag="ps_mlp", name="ps_mlp")
                        for k, (w, rhs) in enumerate(terms):
                            if rhs is None:
                                rhs_ap = xi[cj]
                            else:
                                rhs_ap = rhs[:, cj * 512:cj * 512 + 512]
                            nc.tensor.matmul(
                                ps[:], t_w[w][:], rhs_ap,
                                start=(k == 0), stop=(k == len(terms) - 1))
                        nc.scalar.activation(
                            r[:, cj * 512:(cj + 1) * 512], ps[:], func,
                            bias=bias[:, 0:1], scale=1.0)
                    return r

                r1 = layer(3, [("w1b", t_xg), ("w1a", None)], t_b[1], relu)
                r2 = layer(2, [("w2r1", r1), ("w2x", None)], t_b[2], relu)
                r3 = layer(1, [("w3r2", r2), ("w3r1", r1), ("w3x", None)],
                           t_b[3], relu)
                layer(0, [("w4r3", r3), ("w4r2", r2), ("w4r1", r1),
                          ("w4x", None)], t_b[4], ident_f)

            def stage_reduce(p):
                t_agg = aggp.tile([128, 512], f32, tag="agg", name="agg")
                AGG_t[p] = t_agg
                r_all = R_t.pop(p)
                for l in range(4):
                    rl = r_all[:, l * FT:(l + 1) * FT]
                    r3d = dataclasses.replace(
                        rl, ap=type(rl.ap)([list(rl.ap[0]), [16, 128], [1, 16]]))
                    nc.vector.tensor_reduce(
                        t_agg[:, 128 * l:128 * l + 128], r3d,
                        axis=mybir.AxisListType.X, op=mx)

            def stage_out(p):
                t_agg = AGG_t.pop(p)
                for l in range(4):
                    nc.gpsimd.dma_start(
                        d_out[64 * l:64 * l + 64, 256 * p:256 * p + 128],
                        t_agg[0:64, 128 * l:128 * l + 128])
                    nc.gpsimd.dma_start(
                        d_out[64 * l:64 * l + 64, 256 * p + 128:256 * p + 256],
                        t_agg[64:128, 128 * l:128 * l + 128])

            # 3-deep software pipeline. Per-engine FIFO per iteration: PE
            # [scores(it), transpose(it-1), mlp(it-2)], ACT [d2 copies(it),
            # idx-cast(it-1), acts(it-2)], DVE [reduce(it-3), sel(it)],
            # gpsimd [gather(it-1), out-dma(it-3)]. The DVE never waits on a
            # same-iteration producer (reduce inputs finished last iteration,
            # sel streams behind the ACT d2 copies), and the gather feeding
            # mlp(p) completes a full iteration before mlp(p) issues.
            for it in range(NPAIR + 3):
                if it < NPAIR:
                    stage_scores(it)
                if 1 <= it <= NPAIR:
                    # keep the id-wrap path ahead of the mlp in the
                    # scheduler's priority heap: a late transpose/gather
                    # serializes every downstream engine
                    with tc.high_priority():
                        stage_transpose(it - 1)
                        stage_cidx(it - 1)
                        stage_gather(it - 1)
                if 3 <= it:
                    stage_reduce(it - 3)
                if it < NPAIR:
                    stage_sel(it)
                if 3 <= it:
                    stage_out(it - 3)
                if 2 <= it and it - 2 < NPAIR:
                    stage_mlp(it - 2)

    nc.compile()
    return nc


def host_prep(x, pos, W_first, b_first, W_mid1, b_mid1, W_mid2, b_mid2,
              W_last, b_last):
    """Build the 8 per-core input maps (pure marshalling: slicing/stacking)."""
    x = np.ascontiguousarray(np.asarray(x, np.float32))
    pos = np.ascontiguousarray(np.asarray(pos, np.float32))

    def blk(w):
        o = np.zeros((128, 128), np.float32)
        o[:64, :64] = w
        o[64:, 64:] = w
        return o

    Wf = np.asarray(W_first, np.float32)
    A = Wf[0:64] - Wf[128:192]
    Bm = Wf[64:128] + Wf[128:192]
    W1 = np.asarray(W_mid1, np.float32)
    W2 = np.asarray(W_mid2, np.float32)
    W3 = np.asarray(W_last, np.float32)
    weights = {
        "w1b": blk(Bm), "w1a": blk(A),
        "w2r1": blk(W1[0:64]), "w2x": blk(W1[64:128]),
        "w3r2": blk(W2[0:64]), "w3r1": blk(W2[64:128]), "w3x": blk(W2[128:192]),
        "w4r3": blk(W3[0:64]), "w4r2": blk(W3[64:128]), "w4r1": blk(W3[128:192]),
        "w4x": blk(W3[192:256]),
    }
    for n in ("w2r1", "w3r2", "w3r1", "w4r3", "w4r2", "w4r1"):
        weights[n] = np.ascontiguousarray(weights[n].astype(ml_dtypes.bfloat16))
    biases = {f"b{l}": np.ascontiguousarray(
        np.concatenate([bv, bv]).astype(np.float32)[:, None])
        for l, bv in ((1, b_first), (2, b_mid1), (3, b_mid2), (4, b_last))}

    bfnp = ml_dtypes.bfloat16

    def split3(v):
        h = v.astype(bfnp).astype(np.float32)
        m = (v - h).astype(bfnp).astype(np.float32)
        lo = (v - h - m).astype(bfnp).astype(np.float32)
        return h, m, lo

    intc = np.zeros((128, 112), np.uint32)
    intc[:, 0:64] = np.uint32(0xFFFFFFFF) << np.uint32(LB)   # value mask
    intc[:, 64:80] = (1 << LB) - 1                           # position mask
    intc[:, 80:96] = 3                                       # slot >> 3
    intc[:, 96:112] = 9                                      # << log2(CH)

    in_maps = []
    for c in range(8):
        b, h = c // 2, c % 2
        qs = h * NQ
        p = pos[b]
        cn = (p * p).sum(-1).astype(np.float32)
        # bf16 triple-split: K=30 bf16 matmul reproduces -|q-c|^2 to ~6e-6
        Qh, Qm, Ql = split3((2.0 * p).astype(np.float32))   # [N, 3]
        Chs, Cms, Cls = split3(p)
        cnh, cnm, cnl = split3(cn)
        neg1 = -np.ones((3, N), np.float32)
        qaug_f = np.concatenate(
            [Qh.T, Qh.T, Qm.T, Qh.T, Ql.T, Qm.T, Qm.T, Ql.T, neg1,
             np.stack([cnh, cnm, cnl])], 0)
        caug_f = np.concatenate(
            [Chs.T, Cms.T, Chs.T, Cls.T, Chs.T, Cms.T, Cls.T, Cms.T,
             np.stack([cnh, cnm, cnl]), neg1], 0)          # [30, N]
        # roll the candidate order by qs so the self block of query tile t is
        # always at columns [128*t, 128*t+128) regardless of the core's half
        caug = np.ascontiguousarray(np.roll(caug_f, -qs, axis=1).astype(bfnp))
        qaug = np.ascontiguousarray(qaug_f[:, qs:qs + NQ].astype(bfnp))
        xt = np.ascontiguousarray(x[b].T)                   # [64, 4096]
        xtf = np.ascontiguousarray(
            np.roll(np.concatenate([xt, xt], 0), -qs, axis=1))
        xtq = np.ascontiguousarray(xt[:, qs:qs + NQ])
        v = xtq.reshape(64, NPAIR, 2, 128)
        xtqf = np.ascontiguousarray(
            np.concatenate([v[:, :, 0, :], v[:, :, 1, :]], 0).reshape(128, NQ // 2))
        # self-block penalty: the kernel subtracts 2^20 on the diagonal of the
        # 128-col block at qs + 128*t; queries are rows qs.. so the identity
        # works for every tile with the same [128,128] constants
        nbig = np.ascontiguousarray((-np.eye(128) * (2.0 ** 20)).astype(bfnp))
        selfb = np.zeros((128, 2048), np.float32)
        for v in range(4):
            selfb[:, 512 * v + 128 * v:512 * v + 128 * v + 128] = np.eye(128)
        m = dict(caug=caug, qaug=qaug, xtf=xtf, xtqf=xtqf, xtq=xtq,
                 ident=np.eye(128, dtype=np.float32), nbig=nbig,
                 selfb=np.ascontiguousarray(selfb.astype(bfnp)),
                 intc=np.ascontiguousarray(intc), **weights, **biases)
        in_maps.append(m)
    return in_maps


_NC_CACHE = {}


def _get_nc():
    if "nc" not in _NC_CACHE:
        _NC_CACHE["nc"] = build_nc()
    return _NC_CACHE["nc"]


def kernel(**inputs) -> np.ndarray:
    in_maps = host_prep(**inputs)
    nc = _get_nc()
    res = bass_utils.run_bass_kernel_spmd(nc, in_maps, list(range(8)))
    out = np.empty((B, N, OUTF), np.float32)
    for c in range(8):
        b, h = c // 2, c % 2
        out[b, h * NQ:(h + 1) * NQ, :] = res.results[c]["out"].T
    return out
